# revision 1
# baseline (speedup 1.0000x reference)
"""Trainium2 Bass kernel for nn_DeformConv2d_86105504350808.

The reference's offset/mask convolutions are dead code (computed then
discarded), so the output is exactly a 3x3 stride-1 pad-1 conv with bias:
    out = conv2d(x, weight, pad=1) + bias
x: (32, 256, 64, 64) f32, weight: (256, 256, 3, 3) f32, bias: (256,) f32.

Strategy (8 NeuronCores, data-parallel over batch: 4 images/core):
  - Host: pad x to (b, 256, 66, 66) and cast to fp16 so every
    (image, ci-chunk) is one fully contiguous DMA into SBUF [128, 66, 66]
    (scattered 128B-row DMA writes measured 3x slower than contiguous).
    Weights host-transposed to [ci, (kh*3+kw)*256+co] fp16; fp16 (not bf16)
    because it runs at the same PE rate with 8x lower error.
  - Device: implicit GEMM. For each 4-row output strip (N=256 free dim --
    measured best ns/column on this hardware) and each co-chunk, accumulate
    18 matmuls (9 taps x 2 ci-chunks) of [K=128]x[128x256] fp16 into one
    PSUM tile, evict with per-partition bias add on VectorE into an 8-row
    SBUF tile, DMA out with 2KB-contiguous runs.
  - fp32 accumulation in PSUM throughout; output fp32.
"""

from contextlib import nullcontext

import numpy as np

import concourse.bass as bass  # noqa: F401  (registers engines)
import concourse.tile as tile
from concourse import bacc, mybir
from concourse.bass_utils import run_bass_kernel_spmd

B_FULL = 32
N_CORES = 8
B_SHARD = B_FULL // N_CORES  # 4
CIN = 256
COUT = 256
H = W = 64
PADH, PADW = H + 2, W + 2  # 66
NTAPS = 9
ROWS_PER_STRIP = 4  # N=256 moving free dim: best measured PE rate
OUT_ROWS = 8  # rows per output tile / DMA (2 strips, 2KB runs)
CI_CHUNKS = CIN // 128
CO_CHUNKS = COUT // 128

_cache = {}


def _build():
    f16 = mybir.dt.float16
    f32 = mybir.dt.float32

    nc = bacc.Bacc("TRN2", target_bir_lowering=False, debug=False,
                   num_devices=N_CORES)

    x_d = nc.dram_tensor("x0", [B_SHARD, CIN, PADH, PADW], f16,
                         kind="ExternalInput").ap()
    wt_d = nc.dram_tensor("wt0", [CIN, NTAPS * COUT], f16,
                          kind="ExternalInput").ap()
    bias_d = nc.dram_tensor("bias", [COUT, 1], f32, kind="ExternalInput").ap()
    out_d = nc.dram_tensor("out", [B_SHARD, COUT, H, W], f32,
                           kind="ExternalOutput").ap()

    n_mm_per_group = CI_CHUNKS * NTAPS  # 18

    with tile.TileContext(nc) as tc:
        with (
            tc.tile_pool(name="weights", bufs=1) as wpool,
            tc.tile_pool(name="xpad", bufs=2) as xpool,
            tc.tile_pool(name="outs", bufs=4) as opool,
            tc.tile_pool(name="psum", bufs=8, space="PSUM") as ppool,
        ):
            # Resident weights: [128, 2*2304]; ci chunk c in cols
            # [c*2304, (c+1)*2304); within that, col = tap*256 + o*128 + co.
            w_tile = wpool.tile([128, CI_CHUNKS * NTAPS * COUT], f16, tag="w")
            for c in range(CI_CHUNKS):
                nc.sync.dma_start(
                    w_tile[:, c * NTAPS * COUT:(c + 1) * NTAPS * COUT],
                    wt_d[c * 128:(c + 1) * 128, :])

            bias_sb = wpool.tile([128, CO_CHUNKS], f32, tag="bias")
            for o in range(CO_CHUNKS):
                nc.sync.dma_start(bias_sb[:, o:o + 1],
                                  bias_d[o * 128:(o + 1) * 128, :])

            for b in range(B_SHARD):
                xp = {}
                for c in range(CI_CHUNKS):
                    xt = xpool.tile([128, PADH, PADW], f16, name=f"xt{c}",
                                    tag=f"x{c}")
                    bounds = [PADH * i // 3 for i in range(4)]
                    for r0, r1 in zip(bounds, bounds[1:]):
                        nc.sync.dma_start(
                            xt[:, r0:r1, :],
                            x_d[b, c * 128:(c + 1) * 128, r0:r1, :])
                    xp[c] = xt

                ot = {}
                for h0 in range(0, H, ROWS_PER_STRIP):
                    for o in range(CO_CHUNKS):
                        ps = ppool.tile([128, ROWS_PER_STRIP, W], f32)
                        k = 0
                        for c in range(CI_CHUNKS):
                            for kh in range(3):
                                for kw in range(3):
                                    tap = kh * 3 + kw
                                    col = (c * NTAPS * COUT + tap * COUT
                                           + o * 128)
                                    lhsT = w_tile[:, col:col + 128]
                                    rhs = xp[c][
                                        :, h0 + kh:h0 + kh + ROWS_PER_STRIP,
                                        kw:kw + W]
                                    nc.tensor.matmul(
                                        ps[:], lhsT, rhs,
                                        start=(k == 0),
                                        stop=(k == n_mm_per_group - 1))
                                    k += 1
                        r_in_tile = h0 % OUT_ROWS
                        if r_in_tile == 0:
                            ot[o] = opool.tile([128, OUT_ROWS, W], f32,
                                               name=f"ot{o}", tag=f"ot{o}")
                        nc.vector.tensor_scalar_add(
                            ot[o][:, r_in_tile:r_in_tile + ROWS_PER_STRIP, :],
                            ps[:], bias_sb[:, o:o + 1])
                        if r_in_tile + ROWS_PER_STRIP == OUT_ROWS:
                            h_base = h0 + ROWS_PER_STRIP - OUT_ROWS
                            nc.sync.dma_start(
                                out_d[b, o * 128:(o + 1) * 128,
                                      h_base:h_base + OUT_ROWS, :],
                                ot[o][:])

    nc.compile()
    return nc


def _prep_inputs(x, weight, bias):
    x = np.asarray(x, dtype=np.float32)
    weight = np.asarray(weight, dtype=np.float32)
    bias = np.asarray(bias, dtype=np.float32)
    # [co, ci, kh, kw] -> [ci, (kh*3+kw)*256 + co]
    wt = np.ascontiguousarray(
        weight.transpose(1, 2, 3, 0).reshape(CIN, NTAPS * COUT)
    ).astype(np.float16)
    bias2 = np.ascontiguousarray(bias.reshape(COUT, 1))

    x16 = x.astype(np.float16)
    xpad = np.zeros((B_FULL, CIN, PADH, PADW), dtype=np.float16)
    xpad[:, :, 1:1 + H, 1:1 + W] = x16

    in_maps = []
    for i in range(N_CORES):
        in_maps.append({
            "x0": np.ascontiguousarray(xpad[i * B_SHARD:(i + 1) * B_SHARD]),
            "wt0": wt,
            "bias": bias2,
        })
    return in_maps


def kernel(x, weight, bias, offset_w=None, offset_b=None, mask_w=None,
           mask_b=None, **_unused):
    """Full (unsharded) inputs in, full (32,256,64,64) f32 output out.

    offset/mask tensors are accepted but unused: in the reference they are
    computed and then discarded, so they do not affect the output.
    """
    if "nc" not in _cache:
        _cache["nc"] = _build()
    nc = _cache["nc"]
    in_maps = _prep_inputs(x, weight, bias)
    res = run_bass_kernel_spmd(nc, in_maps, core_ids=list(range(N_CORES)))
    out = np.concatenate([res.results[i]["out"] for i in range(N_CORES)],
                         axis=0)
    return out.astype(np.float32, copy=False)
